# revision 1
# baseline (speedup 1.0000x reference)
"""DecoderPooler kernel for Trainium2 (Bass), 8-core data-parallel.

Problem: given hidden_state [16, 4096, 1024] f32 and attention_mask
[16, 4096] int32 (contiguous prefix of ones), return the hidden vector at
the last valid position of each sequence: out[b] = hidden[b, sum(mask[b])-1].

Strategy: shard the batch dim (16) across 8 cores, 2 sequences/core.
Each core reads only its mask rows (~33 KB) and the two needed H-vectors
(8 KB) from HBM — never the other 32 MB of its hidden_state shard:

  1. The host lays each core's two mask rows out as a [128, 65] f32 tile
     (sequence b on partitions [64b, 64b+64), 64 elements per partition,
     plus one trailing column baking the constant b*S - 1 into partition
     64b), so that
  2. a 65-element DVE reduce + a [128,1]x[128,2] PE matmul against a 0/1
     group-selector yields idx[b] = b*S + len_b - 1 on PSUM,
  3. DVE casts the two sums to int32 in SBUF, SP loads them into sequencer
     registers, and issues two register-dynamically-addressed DRAM->DRAM
     DMAs copying hidden[idx[b], :] straight into the output rows.

Raw Bass (no TileContext): the kernel is a short serial chain, and
explicit semaphores keep the tail free of the multi-sem drain that
overflows this walrus build's per-instruction sync-wait limit.
"""

import numpy as np

import concourse.bass as bass
import concourse.mybir as mybir
from concourse.bass_utils import run_bass_kernel_spmd

B, S, H = 16, 4096, 1024
N_CORES = 8
B_PER = B // N_CORES  # 2 sequences per core
PARTS = 64  # partitions per sequence; B_PER*PARTS = 128
CHUNK = S // PARTS  # 64 mask elements per partition

_NC_CACHE = None


def build_bass(reps: int = 1) -> bass.Bass:
    """Per-core program: gather the last valid token of B_PER sequences.

    reps>1 repeats the chain serially (same tiles, cumulative semaphore
    thresholds) — used only for on-device timing by delta: the per-rep
    increment of wall time is the HW kernel latency, with host/RPC/launch
    overhead cancelled out.
    """
    nc = bass.Bass()
    # flat [B_PER*S, H] view of this core's hidden_state shard
    hidden = nc.declare_dram_parameter(
        "hidden", [B_PER * S, H], mybir.dt.float32, isOutput=False
    )
    # host-prepped [128, 65] f32 mask layout (see module docstring)
    mask = nc.declare_dram_parameter(
        "mask", [B_PER * PARTS, CHUNK + 1], mybir.dt.float32, isOutput=False
    )
    out = nc.declare_dram_parameter("out", [B_PER, H], mybir.dt.float32, isOutput=True)

    with (
        nc.sbuf_tensor([B_PER * PARTS, CHUNK + 1], mybir.dt.float32) as work,
        nc.sbuf_tensor([B_PER * PARTS, 1], mybir.dt.float32) as partial,
        nc.sbuf_tensor([B_PER * PARTS, B_PER], mybir.dt.float32) as sel,
        nc.sbuf_tensor([1, B_PER], mybir.dt.int32) as idx,
        nc.psum_tensor([1, B_PER], mybir.dt.float32) as psum,
        nc.semaphore() as dma_sem,
        nc.semaphore() as s_sem,
        nc.semaphore() as v_sem,
        nc.semaphore() as pe_sem,
        nc.Block() as block,
    ):

        @block.sync
        def _(sync):
            r0 = sync.alloc_register("r0")
            for i in range(reps):
                sync.dma_start(out=work[:], in_=mask[:]).then_inc(dma_sem, 16)
                # idx values ready in SBUF
                sync.wait_ge(v_sem, 2 * (i + 1))
                sync.reg_load(r0, idx[0:1, 0:1])
                # donate: the snap aliases the register instead of allocating
                # a fresh snapshot register per rep (the DMA descriptor
                # captures the value at issue, so reuse next rep is safe
                # behind the s_sem wait)
                v0 = sync.snap(r0, donate=True)
                sync.dma_start(
                    out=out[0:1, :], in_=hidden[bass.ds(v0, 1), :]
                ).then_inc(s_sem, 16)
                # don't let the program retire (or the next rep start)
                # with either store still in flight (scalar's included)
                sync.wait_ge(s_sem, 32 * (i + 1))

        @block.scalar
        def _(scalar):
            # second output row handled by ACT's sequencer in parallel with SP
            r1 = scalar.alloc_register("r1")
            for i in range(reps):
                scalar.wait_ge(v_sem, 2 * (i + 1))
                scalar.reg_load(r1, idx[0:1, 1:2])
                v1 = scalar.snap(r1, donate=True)
                scalar.dma_start(
                    out=out[1:2, :], in_=hidden[bass.ds(v1, 1), :]
                ).then_inc(s_sem, 16)
                scalar.wait_ge(s_sem, 32 * (i + 1))

        @block.vector
        def _(vector):
            # 0/1 selector: sel[p, b] = 1 iff partition p belongs to seq b.
            # Written once as disjoint regions; PE reads are ordered behind
            # v_sem via DVE's in-order queue.
            for b in range(B_PER):
                rows = slice(b * PARTS, (b + 1) * PARTS)
                for c in range(B_PER):
                    vector.memset(sel[rows, c : c + 1], 1.0 if b == c else 0.0)
            for i in range(reps):
                vector.wait_ge(dma_sem, 16 * (i + 1))
                # partial[p] = sum(work[p, :]); all values exact ints in f32
                vector.reduce_sum(
                    out=partial[:], in_=work[:], axis=mybir.AxisListType.X
                ).then_inc(v_sem, 1)
                vector.wait_ge(pe_sem, i + 1)
                # psum[0, b] = b*S + len_b - 1; cast exactly to int32
                vector.tensor_copy(idx[:], psum[:]).then_inc(v_sem, 1)

        @block.tensor
        def _(tensor):
            for i in range(reps):
                tensor.wait_ge(v_sem, 2 * i + 1)
                # psum = partial.T @ sel -> [1, B_PER] of per-sequence sums
                nc.tensor.matmul(
                    out=psum[:],
                    lhsT=partial[:],
                    rhs=sel[:],
                    start=True,
                    stop=True,
                ).then_inc(pe_sem, 1)

    return nc


def _get_nc() -> bass.Bass:
    global _NC_CACHE
    if _NC_CACHE is None:
        _NC_CACHE = build_bass()
    return _NC_CACHE


def _prep_mask(mask_rows: np.ndarray) -> np.ndarray:
    """[B_PER, S] 0/1 mask -> [128, 65] f32 tile (see module docstring)."""
    m = np.asarray(mask_rows, dtype=np.float32).reshape(B_PER * PARTS, CHUNK)
    extra = np.zeros((B_PER * PARTS, 1), dtype=np.float32)
    for b in range(B_PER):
        extra[b * PARTS, 0] = b * S - 1
    return np.ascontiguousarray(np.concatenate([m, extra], axis=1))


def _shard_inputs(hidden_state: np.ndarray, attention_mask: np.ndarray):
    in_maps = []
    for c in range(N_CORES):
        lo, hi = c * B_PER, (c + 1) * B_PER
        hs = np.ascontiguousarray(
            hidden_state[lo:hi].reshape(B_PER * S, H), dtype=np.float32
        )
        in_maps.append({"hidden": hs, "mask": _prep_mask(attention_mask[lo:hi])})
    return in_maps


def run(hidden_state: np.ndarray, attention_mask: np.ndarray, **spmd_kwargs):
    """Run on 8 NeuronCores; returns (full_output, BassKernelResults)."""
    nc = _get_nc()
    in_maps = _shard_inputs(np.asarray(hidden_state), np.asarray(attention_mask))
    res = run_bass_kernel_spmd(nc, in_maps, core_ids=list(range(N_CORES)), **spmd_kwargs)
    out = np.concatenate([r["out"] for r in res.results], axis=0)
    return out, res


def kernel(hidden_state: np.ndarray, attention_mask: np.ndarray) -> np.ndarray:
    out, _ = run(hidden_state, attention_mask)
    return out



# revision 4
# speedup vs baseline: 3.2565x; 3.2565x over previous
"""DecoderPooler kernel for Trainium2 (Bass), 8-core data-parallel.

Problem: given hidden_state [16, 4096, 1024] f32 and attention_mask
[16, 4096] int32 (contiguous prefix of ones), return the hidden vector at
the last valid position of each sequence: out[b] = hidden[b, sum(mask[b])-1].

Strategy: shard the batch dim (16) across 8 cores, 2 sequences/core.
Each core reads only its mask rows (~17 KB as bf16) and the two needed
H-vectors (8 KB) from HBM — never the other 32 MB of its hidden_state shard:

  1. The host lays each core's two mask rows out as a [128, 66] bf16 tile:
     sequence b on partitions [64b, 64b+64), 64 mask elements per partition,
     plus two trailing columns baking the gather-row constants as bf16-exact
     addends (-1 on partition 0; +4096 and -1 split across the two columns
     on partition 64, since 4095 itself is not bf16-representable), so that
  2. a 66-element DVE reduce (bf16 in, f32 out — sums stay exact) + a
     [128,1]x[128,2] PE matmul against a 0/1 group-selector yields
     idx[b] = b*S + len_b - 1 on PSUM,
  3. DVE casts the two sums to int32 in SBUF, SP loads them into sequencer
     registers, and issues two register-dynamically-addressed DRAM->DRAM
     DMAs copying hidden[idx[b], :] straight into the output rows (row 1
     handled by ACT's sequencer in parallel).

The serial chain is dominated by the two unavoidable DMA round trips
(mask load ~2.2us, dynamic gather ~2.2us, mostly fixed descriptor-gen /
DGE / completion-semaphore latency); everything between them is ~0.5us.
Variants that shortened the middle (GpSimd XYZWC reduces straight to
int32, PSUM register loads) measured slower on HW (GpSimd reduce is slow
in silicon) or don't compile (walrus rejects TensorLoad from PSUM), so
this keeps the proven engine chain and only shrinks the mask transfer.

Raw Bass (no TileContext): the kernel is a short serial chain, and
explicit semaphores keep the tail free of the multi-sem drain that
overflows this walrus build's per-instruction sync-wait limit.
"""

import numpy as np

import concourse.bass as bass
import concourse.mybir as mybir
from concourse.bass_utils import run_bass_kernel_spmd

B, S, H = 16, 4096, 1024
N_CORES = 8
B_PER = B // N_CORES  # 2 sequences per core
PARTS = 64  # partitions per sequence; B_PER*PARTS = 128
CHUNK = S // PARTS  # 64 mask elements per partition
CCOLS = 2  # trailing constant columns (split bf16-exact addends)

_NC_CACHE = None


def build_bass(reps: int = 1) -> bass.Bass:
    """Per-core program: gather the last valid token of B_PER sequences.

    reps>1 repeats the chain serially (same tiles, cumulative semaphore
    thresholds) — used only for on-device timing by delta: the per-rep
    increment of wall time is the HW kernel latency, with host/RPC/launch
    overhead cancelled out.
    """
    nc = bass.Bass()
    # flat [B_PER*S, H] view of this core's hidden_state shard
    hidden = nc.declare_dram_parameter(
        "hidden", [B_PER * S, H], mybir.dt.float32, isOutput=False
    )
    # host-prepped [128, 66] bf16 mask layout (see module docstring)
    mask = nc.declare_dram_parameter(
        "mask", [B_PER * PARTS, CHUNK + CCOLS], mybir.dt.bfloat16, isOutput=False
    )
    out = nc.declare_dram_parameter("out", [B_PER, H], mybir.dt.float32, isOutput=True)

    with (
        nc.sbuf_tensor([B_PER * PARTS, CHUNK + CCOLS], mybir.dt.bfloat16) as work,
        nc.sbuf_tensor([B_PER * PARTS, 1], mybir.dt.float32) as partial,
        nc.sbuf_tensor([B_PER * PARTS, B_PER], mybir.dt.float32) as sel,
        nc.sbuf_tensor([1, B_PER], mybir.dt.int32) as idx,
        nc.psum_tensor([1, B_PER], mybir.dt.float32) as psum,
        nc.semaphore() as dma_sem,
        nc.semaphore() as s_sem,
        nc.semaphore() as v_sem,
        nc.semaphore() as pe_sem,
        nc.Block() as block,
    ):

        @block.sync
        def _(sync):
            r0 = sync.alloc_register("r0")
            for i in range(reps):
                sync.dma_start(out=work[:], in_=mask[:]).then_inc(dma_sem, 16)
                # idx values ready in SBUF
                sync.wait_ge(v_sem, 2 * (i + 1))
                sync.reg_load(r0, idx[0:1, 0:1])
                # donate: the snap aliases the register instead of allocating
                # a fresh snapshot register per rep (the DMA descriptor
                # captures the value at issue, so reuse next rep is safe
                # behind the s_sem wait)
                v0 = sync.snap(r0, donate=True)
                sync.dma_start(
                    out=out[0:1, :], in_=hidden[bass.ds(v0, 1), :]
                ).then_inc(s_sem, 16)
                # don't let the program retire (or the next rep start)
                # with either store still in flight (scalar's included)
                sync.wait_ge(s_sem, 32 * (i + 1))

        @block.scalar
        def _(scalar):
            # second output row handled by ACT's sequencer in parallel with SP
            r1 = scalar.alloc_register("r1")
            for i in range(reps):
                scalar.wait_ge(v_sem, 2 * (i + 1))
                scalar.reg_load(r1, idx[0:1, 1:2])
                v1 = scalar.snap(r1, donate=True)
                scalar.dma_start(
                    out=out[1:2, :], in_=hidden[bass.ds(v1, 1), :]
                ).then_inc(s_sem, 16)
                scalar.wait_ge(s_sem, 32 * (i + 1))

        @block.vector
        def _(vector):
            # 0/1 selector: sel[p, b] = 1 iff partition p belongs to seq b.
            # Written once as disjoint regions; PE reads are ordered behind
            # v_sem via DVE's in-order queue.
            for b in range(B_PER):
                rows = slice(b * PARTS, (b + 1) * PARTS)
                for c in range(B_PER):
                    vector.memset(sel[rows, c : c + 1], 1.0 if b == c else 0.0)
            for i in range(reps):
                vector.wait_ge(dma_sem, 16 * (i + 1))
                # partial[p] = sum(work[p, :]); bf16 0/1 values and constants
                # are exact, accumulation is f32
                vector.reduce_sum(
                    out=partial[:], in_=work[:], axis=mybir.AxisListType.X
                ).then_inc(v_sem, 1)
                vector.wait_ge(pe_sem, i + 1)
                # psum[0, b] = b*S + len_b - 1; cast exactly to int32
                vector.tensor_copy(idx[:], psum[:]).then_inc(v_sem, 1)

        @block.tensor
        def _(tensor):
            for i in range(reps):
                tensor.wait_ge(v_sem, 2 * i + 1)
                # psum = partial.T @ sel -> [1, B_PER] of per-sequence sums
                nc.tensor.matmul(
                    out=psum[:],
                    lhsT=partial[:],
                    rhs=sel[:],
                    start=True,
                    stop=True,
                ).then_inc(pe_sem, 1)

    return nc


def _get_nc() -> bass.Bass:
    global _NC_CACHE
    if _NC_CACHE is None:
        _NC_CACHE = build_bass()
    return _NC_CACHE


def _prep_mask(mask_rows: np.ndarray) -> np.ndarray:
    """[B_PER, S] 0/1 mask -> [128, 66] bf16 tile (see module docstring)."""
    bf16 = mybir.dt.np(mybir.dt.bfloat16)
    m = np.asarray(mask_rows, dtype=np.float32).reshape(B_PER * PARTS, CHUNK)
    extra = np.zeros((B_PER * PARTS, CCOLS), dtype=np.float32)
    # seq 0 gather row: len0 - 1          -> bake -1
    # seq 1 gather row: S + len1 - 1      -> bake +4096 and -1 (4095 is not
    #                                        bf16-exact; the split addends are)
    extra[0, 0] = -1.0
    extra[PARTS, 0] = float(S)
    extra[PARTS, 1] = -1.0
    return np.ascontiguousarray(np.concatenate([m, extra], axis=1).astype(bf16))


def _shard_inputs(hidden_state: np.ndarray, attention_mask: np.ndarray):
    in_maps = []
    for c in range(N_CORES):
        lo, hi = c * B_PER, (c + 1) * B_PER
        hs = np.ascontiguousarray(
            hidden_state[lo:hi].reshape(B_PER * S, H), dtype=np.float32
        )
        in_maps.append({"hidden": hs, "mask": _prep_mask(attention_mask[lo:hi])})
    return in_maps


def run(hidden_state: np.ndarray, attention_mask: np.ndarray, **spmd_kwargs):
    """Run on 8 NeuronCores; returns (full_output, BassKernelResults)."""
    nc = _get_nc()
    in_maps = _shard_inputs(np.asarray(hidden_state), np.asarray(attention_mask))
    res = run_bass_kernel_spmd(nc, in_maps, core_ids=list(range(N_CORES)), **spmd_kwargs)
    out = np.concatenate([r["out"] for r in res.results], axis=0)
    return out, res


def kernel(hidden_state: np.ndarray, attention_mask: np.ndarray) -> np.ndarray:
    out, _ = run(hidden_state, attention_mask)
    return out
